# revision 35
# baseline (speedup 1.0000x reference)
"""Trainium2 Bass kernel for nn_AttentionBlock (GroupNorm + MHA + proj + residual).

Sharding: data-parallel over batch (16 batches -> 2 per core x 8 cores).
Weights replicated. Each core computes its 2 batches fully; host gathers.

Per-batch dataflow on a core (c=512, t=1024, H=8 heads, dh=64, 32 groups):
  x [512,1024] -> GroupNorm (bn_stats + tiny mask-matmul partition reduce,
                  no DRAM bounce) -> xn (bf16)
  qk = Wqk_reordered @ xn   (8 o-tiles; pair-ordered so head-pairs share tiles)
  vT = xn^T @ Wv^T          (v produced transposed: [s, c_v], ones col per head)
  per head-pair: logitsT[s,t] = k^T q (64-row stationary per head)
                 wT = exp(logitsT)  (ScalarE, PSUM->SBUF)
                 attnRaw[65,t] = vAugT^T @ wT  (row 64 = softmax denominator)
                 rec = recip(denom) -> DMA partition-broadcast -> evac-mult
  out = w_proj @ attn + b_proj + x  (residual x re-DMA'd from DRAM)

Schedule: software-pipelined so the PE never idles: per attention group the
next pair's logits (4 units) + filler matmul groups (batch-1 qkv during
batch-0 attention, proj afterwards) are woven in.
"""

import os
import sys

os.environ.setdefault("MYCRO_LOCAL_CACHE", "1")
for _p in ("/root/.axon_site", "/root/.axon_site/_ro/trn_rl_repo",
           "/root/.axon_site/_ro/pypackages", "/opt/trn_rl_repo"):
    if os.path.isdir(_p) and _p not in sys.path:
        sys.path.append(_p)

import numpy as np

from concourse import bass, bacc, tile, mybir
from concourse._compat import get_trn_type
from concourse.bass_utils import run_bass_kernel_spmd

F32 = mybir.dt.float32
BF16 = mybir.dt.bfloat16
MUL = mybir.AluOpType.mult
ADD = mybir.AluOpType.add

N_CORES = 8
B, C, HH, WW = 16, 512, 32, 32
T = HH * WW            # 1024
NHEADS = 8
DH = C // NHEADS       # 64
NGROUPS = 32
GSIZE = C // NGROUPS   # 16 channels per group
EPS = 1e-5
BPC = B // N_CORES     # batches per core = 2
P = 128
NPAIR = NHEADS // 2    # 4 head pairs
CT = C // P            # 4 channel tiles
OT = (2 * C) // P      # 8 qk output tiles
ST = T // P            # 8 s-tiles
TH = T // 512          # 2 t-halves

LAST_RESULTS = None


def _bc_ap(ap, nparts):
    """Broadcast an AP along a new leading partition dim of size nparts."""
    return bass.AP(tensor=ap.tensor, offset=ap.offset,
                   ap=[[0, nparts]] + [list(d) for d in ap.ap])


def build_nc():
    nc = bacc.Bacc(get_trn_type() or "TRN2", target_bir_lowering=False,
                   debug=False)

    xs_d = nc.dram_tensor("xs", [BPC, C, T], F32, kind="ExternalInput")
    wqkT_d = nc.dram_tensor("wqkT", [C, 2 * C], BF16, kind="ExternalInput")
    bqkT_d = nc.dram_tensor("bqkT", [P, OT], F32, kind="ExternalInput")
    wvT_d = nc.dram_tensor("wvT", [C, C], BF16, kind="ExternalInput")
    bvA_d = nc.dram_tensor("bvA", [NHEADS * 65], F32, kind="ExternalInput")
    wpT_d = nc.dram_tensor("wpT", [C, C], BF16, kind="ExternalInput")
    bpT_d = nc.dram_tensor("bpT", [P, CT], F32, kind="ExternalInput")
    gnsb_d = nc.dram_tensor("gnsb", [P, 2 * CT], F32, kind="ExternalInput")
    mred_d = nc.dram_tensor("mred", [CT, P, NGROUPS], F32, kind="ExternalInput")
    mbc_d = nc.dram_tensor("mbc", [CT, NGROUPS, P], F32, kind="ExternalInput")
    out_d = nc.dram_tensor("out", [BPC, C, T], F32, kind="ExternalOutput")
    NGT = P // GSIZE       # groups per 128-channel tile = 8

    from contextlib import ExitStack
    with ExitStack() as ctx:
        tc = ctx.enter_context(tile.TileContext(nc))
        cpool = ctx.enter_context(tc.tile_pool(name="const", bufs=1))
        xpool = ctx.enter_context(tc.tile_pool(name="xp", bufs=5))
        xnpool = ctx.enter_context(tc.tile_pool(name="xnp", bufs=8))
        qkpool = ctx.enter_context(tc.tile_pool(name="qkp", bufs=12))
        vtpool = ctx.enter_context(tc.tile_pool(name="vtp", bufs=16))
        wtpool = ctx.enter_context(tc.tile_pool(name="wtp", bufs=24))
        attnpool = ctx.enter_context(tc.tile_pool(name="attnp", bufs=7))
        outpool = ctx.enter_context(tc.tile_pool(name="outp", bufs=4))
        xrpool = ctx.enter_context(tc.tile_pool(name="xrp", bufs=5))
        smallpool = ctx.enter_context(tc.tile_pool(name="smallp", bufs=3))
        dramp = ctx.enter_context(tc.tile_pool(name="dramp", bufs=2, space="DRAM"))
        ps_mm = ctx.enter_context(tc.tile_pool(name="ps_mm", bufs=2, space="PSUM"))
        ps_lg = ctx.enter_context(tc.tile_pool(name="ps_lg", bufs=2, space="PSUM"))
        ps_at = ctx.enter_context(tc.tile_pool(name="ps_at", bufs=2, space="PSUM"))

        dma = nc.default_dma_engine

        def load_x(b, split=1):
            xs = []
            for j in range(CT):
                xt = xpool.tile([P, T], F32, tag="x")
                w = T // split
                for c in range(split):
                    dma.dma_start(xt[:, w * c:w * (c + 1)],
                                  xs_d[b, P * j:P * (j + 1),
                                       w * c:w * (c + 1)])
                xs.append(xt)
            return xs

        # DMA issue order = SP queue order: x0 first, GN consts, qk weights,
        # then x1 and the rest (wpT last: needed only at proj time)
        x0 = load_x(0, split=2)
        gnsb_sb = cpool.tile([P, 2 * CT], F32, tag="gnsb")
        dma.dma_start(gnsb_sb[:], gnsb_d[:])
        mred4_sb, mbc4_sb = [], []
        for j in range(CT):
            m = cpool.tile([P, NGROUPS], F32, tag=f"mred{j}")
            dma.dma_start(m[:], mred_d[j])
            mred4_sb.append(m)
            m2 = cpool.tile([NGROUPS, P], F32, tag=f"mbc{j}")
            dma.dma_start(m2[:], mbc_d[j])
            mbc4_sb.append(m2)
        wqkT_sb = []
        for k in range(CT):
            w = cpool.tile([P, 2 * C], BF16, tag=f"wqkT{k}")
            dma.dma_start(w[:], wqkT_d[P * k:P * (k + 1), :])
            wqkT_sb.append(w)
        bqk_sb = cpool.tile([P, OT], F32, tag="bqk")
        dma.dma_start(bqk_sb[:], bqkT_d[:])
        wvT_sb = []
        for k in range(CT):
            w = cpool.tile([P, C], BF16, tag=f"wvT{k}")
            dma.dma_start(w[:], wvT_d[P * k:P * (k + 1), :])
            wvT_sb.append(w)
        bv_bc = cpool.tile([P, NHEADS * 65], F32, tag="bv")
        dma.dma_start(bv_bc[:], _bc_ap(bvA_d.ap(), P))
        x1 = load_x(1, split=2)
        bp_sb = cpool.tile([P, CT], F32, tag="bp")
        dma.dma_start(bp_sb[:], bpT_d[:])
        wpT_sb = []
        for k in range(CT):
            w = cpool.tile([P, C], BF16, tag=f"wpT{k}")
            dma.dma_start(w[:], wpT_d[P * k:P * (k + 1), :])
            wpT_sb.append(w)
        zero_b = cpool.tile([P, 1], F32, tag="zerob")
        nc.vector.memset(zero_b[:], 0.0)
        magic_sb = cpool.tile([NGROUPS, 1], mybir.dt.int32, tag="magic")
        nc.vector.memset(magic_sb[:], 0x5F3759DF)
        c15_sb = cpool.tile([NGROUPS, 1], F32, tag="c15")
        nc.vector.memset(c15_sb[:], 1.5)

        # ---------- GroupNorm: combined 32-group stage, all-DVE rsqrt ------
        I32 = mybir.dt.int32

        def gn_stats(xt, vals_list):
            bst = smallpool.tile([P, 2, 6], F32, tag="bst")
            nc.vector.bn_stats(out=bst[:, 0, :], in_=xt[:, 0:512])
            nc.vector.bn_stats(out=bst[:, 1, :], in_=xt[:, 512:1024])
            mv = smallpool.tile([P, 3], F32, tag="mv", bufs=5)
            nc.vector.bn_aggr(out=mv[:, 0:2], in_=bst[:])
            nc.vector.tensor_mul(mv[:, 2:3], mv[:, 0:1], mv[:, 0:1])
            vals_list.append(mv)

        def gn_coeffs(vals_list):
            """Reduce all 32 groups, rsqrt via bit-trick + 2 Newton (DVE only).
            Returns amu [32, 2] = (a_g, mu_g)."""
            gp = ps_mm.tile([P, 512], F32, tag="psmm")
            for j in range(CT):
                nc.tensor.matmul(gp[0:NGROUPS, 0:3], mred4_sb[j][:],
                                 vals_list[j][:],
                                 start=(j == 0), stop=(j == CT - 1))
            gs = smallpool.tile([NGROUPS, 3], F32, tag="gs", bufs=3)
            nc.vector.tensor_copy(gs[:], gp[0:NGROUPS, 0:3])
            gg = smallpool.tile([NGROUPS, 7], F32, tag="gg", bufs=3)
            # cols: 0=mu 1=sum(var)+sum(mu^2) 2=v(+eps) 3=h 5=u 6=e
            nc.vector.tensor_scalar_mul(gg[:, 0:1], gs[:, 0:1], 1.0 / GSIZE)
            nc.vector.tensor_add(gg[:, 1:2], gs[:, 1:2], gs[:, 2:3])
            nc.vector.tensor_mul(gg[:, 2:3], gg[:, 0:1], gg[:, 0:1])
            nc.vector.scalar_tensor_tensor(
                out=gg[:, 2:3], in0=gg[:, 1:2], scalar=1.0 / GSIZE,
                in1=gg[:, 2:3], op0=MUL, op1=mybir.AluOpType.subtract)
            nc.vector.tensor_scalar_add(gg[:, 2:3], gg[:, 2:3], EPS)
            nc.vector.tensor_scalar_mul(gg[:, 3:4], gg[:, 2:3], 0.5)  # h=v/2
            gi = smallpool.tile([NGROUPS, 1], I32, tag="gi")
            nc.vector.tensor_scalar(
                out=gi[:], in0=gg[:, 2:3].bitcast(I32), scalar1=1,
                scalar2=None, op0=mybir.AluOpType.logical_shift_right)
            amu = smallpool.tile([NGROUPS, 2], F32, tag="amu")
            y = amu[:, 0:1]
            nc.vector.tensor_sub(y.bitcast(I32), magic_sb[:], gi[:])
            for _ in range(2):  # y *= 1.5 - h*y^2
                nc.vector.tensor_mul(gg[:, 5:6], y, y)
                nc.vector.tensor_mul(gg[:, 5:6], gg[:, 5:6], gg[:, 3:4])
                nc.vector.tensor_sub(gg[:, 6:7], c15_sb[:], gg[:, 5:6])
                nc.vector.tensor_mul(y, y, gg[:, 6:7])
            nc.vector.tensor_copy(amu[:, 1:2], gg[:, 0:1])
            return amu

        def gn_abj(j, amu):
            bp2 = ps_mm.tile([P, 512], F32, tag="psmm")
            nc.tensor.matmul(bp2[:, 0:2], mbc4_sb[j][:], amu[:],
                             start=True, stop=True)
            abj = smallpool.tile([P, 2], F32, tag="abj", bufs=6)
            nc.vector.tensor_mul(abj[:, 0:1], bp2[:, 0:1],
                                 gnsb_sb[:, 2 * j:2 * j + 1])
            tmpm = smallpool.tile([P, 1], F32, tag="tmpm")
            nc.vector.tensor_mul(tmpm[:], bp2[:, 1:2], abj[:, 0:1])
            nc.vector.tensor_sub(abj[:, 1:2],
                                 gnsb_sb[:, 2 * j + 1:2 * j + 2], tmpm[:])
            return abj

        def gn_norm(xt, abj, xn_out, eng):
            if eng is nc.scalar:
                nc.scalar.activation(
                    xn_out[:], xt[:], mybir.ActivationFunctionType.Identity,
                    bias=abj[:, 1:2], scale=abj[:, 0:1])
            else:
                eng.tensor_scalar(
                    out=xn_out[:], in0=xt[:], scalar1=abj[:, 0:1],
                    scalar2=abj[:, 1:2], op0=MUL, op1=ADD)

        def gn_apply(xt, j, amu, xn_out, eng=None):
            gn_norm(xt, gn_abj(j, amu), xn_out, eng or nc.vector)

        def emit_gn(x_sb, xn_sb):
            vals_list = []
            for j in range(CT):
                gn_stats(x_sb[j], vals_list)
            amu = gn_coeffs(vals_list)
            abjs = [gn_abj(j, amu) for j in range(CT)]
            engs = [nc.vector, nc.gpsimd, nc.scalar, nc.gpsimd]
            for j in range(CT):
                xn = xnpool.tile([P, T], BF16, tag="xn")
                xn_sb.append(xn)
                gn_norm(x_sb[j], abjs[j], xn, engs[j])

        # ---------- qkv / proj group emitters (one PSUM group each) ----------
        def qk_group(xn_sb, qk_sb, j, th, act_evac=False):
            def emit():
                if th == 0:
                    qk_sb.append(qkpool.tile([P, T], BF16, tag="qk",
                                             name="qk_t"))
                qk = qk_sb[j]
                ps = ps_mm.tile([P, 512], F32, tag="psmm")
                for k in range(CT):
                    nc.tensor.matmul(
                        ps[:], wqkT_sb[k][:, P * j:P * (j + 1)],
                        xn_sb[k][:, 512 * th:512 * (th + 1)],
                        start=(k == 0), stop=(k == CT - 1))
                if act_evac:
                    nc.scalar.activation(
                        qk[:, 512 * th:512 * (th + 1)], ps[:],
                        mybir.ActivationFunctionType.Identity,
                        bias=bqk_sb[:, j:j + 1])
                else:
                    nc.vector.tensor_scalar_add(
                        qk[:, 512 * th:512 * (th + 1)], ps[:],
                        bqk_sb[:, j:j + 1])
            return emit

        def vt_group(xn_sb, vt_sb, st):
            def emit():
                vt = vtpool.tile([P, NHEADS * 65], BF16, tag="vt")
                vt_sb.append(vt)
                vt3 = vt[:].rearrange("p (h c) -> p h c", h=NHEADS)
                ps = ps_mm.tile([P, 512], F32, tag="psmm")
                for k in range(CT):
                    nc.tensor.matmul(
                        ps[:], xn_sb[k][:, P * st:P * (st + 1)], wvT_sb[k][:],
                        start=(k == 0), stop=(k == CT - 1))
                bv3 = bv_bc[:].rearrange("p (h c) -> p h c", h=NHEADS)
                nc.vector.tensor_add(
                    vt3[:, :, 0:DH],
                    ps[:].rearrange("p (h c) -> p h c", h=NHEADS),
                    bv3[:, :, 0:DH])
                nc.gpsimd.tensor_copy(vt3[:, :, DH:DH + 1],
                                      bv3[:, :, DH:DH + 1])
            return emit

        def proj_steps(ps, at_sb, j, th, ks, stop=False):
            for k in ks:
                nc.tensor.matmul(
                    ps[:], wpT_sb[k][:, P * j:P * (j + 1)],
                    at_sb[k][:, 512 * th:512 * (th + 1)],
                    start=(k == 0), stop=(stop and k == CT - 1))

        def proj_fin(b, ps, at_sb, xr_sb, j, th):
            proj_steps(ps, at_sb, j, th, [CT - 1], stop=True)
            ot = outpool.tile([P, 512], F32, tag="out", name="out_t")
            nc.vector.scalar_tensor_tensor(
                out=ot[:], in0=ps[:], scalar=bp_sb[:, j:j + 1],
                in1=xr_sb[j][:, 512 * th:512 * (th + 1)],
                op0=ADD, op1=ADD)
            dma.dma_start(
                out_d[b, P * j:P * (j + 1), 512 * th:512 * (th + 1)], ot[:])

        def proj_group(b, at_sb, xr_sb, j, th):
            def emit():
                ps = ps_mm.tile([P, 512], F32, tag="psmm")
                proj_steps(ps, at_sb, j, th, range(CT - 1))
                proj_fin(b, ps, at_sb, xr_sb, j, th)
            return emit

        # ---------- attention emitters ----------
        def lg_unit(qk_sb, p_i, hh, st, wts):
            """Logits (2 MMs) + exp for one (head, s-tile). Appends wt tile."""
            def emit():
                qt = qk_sb[2 * p_i]
                kt = qk_sb[2 * p_i + 1]
                lo = DH * hh
                lg = ps_lg.tile([P, T], F32, tag="pslg")
                for th in range(TH):
                    nc.tensor.matmul(
                        lg[:, 512 * th:512 * (th + 1)],
                        kt[lo:lo + DH, P * st:P * (st + 1)],
                        qt[lo:lo + DH, 512 * th:512 * (th + 1)],
                        start=True, stop=True)
                wt = wtpool.tile([P, T], BF16, tag="wt")
                nc.scalar.activation(wt[:], lg[:],
                                     mybir.ActivationFunctionType.Exp,
                                     bias=zero_b[:])
                wts[hh].append(wt)
            return emit

        pending_evac = []

        def attn_group(vt_sb, wts, at, p_i, hh, th, last=False):
            """attn-MM group; raw rows copied out at once (frees the PSUM
            bank), 1/denom broadcast via gpsimd DRAM bounce, divide applied
            in-place later on Pool (pending_evac). For the final pair the
            raw/den copies run on the tail-idle ACT engine instead of DVE."""
            def emit():
                h_abs = 2 * p_i + hh
                pa = ps_at.tile([65, 512], F32, tag="psat")
                for st in range(ST):
                    nc.tensor.matmul(
                        pa[:], vt_sb[st][:, 65 * h_abs:65 * (h_abs + 1)],
                        wts[hh][st][:, 512 * th:512 * (th + 1)],
                        start=(st == 0), stop=(st == ST - 1))
                sl = slice(512 * th, 512 * (th + 1))
                rows = slice(DH * hh, DH * hh + DH)
                den = smallpool.tile([1, 512], F32, tag="den", bufs=2)
                if last:
                    nc.scalar.copy(den[:], pa[DH:DH + 1, :])
                    nc.scalar.copy(at[rows, sl], pa[0:DH, :])
                else:
                    nc.vector.tensor_copy(den[:], pa[DH:DH + 1, :])
                    nc.vector.tensor_copy(at[rows, sl], pa[0:DH, :])
                rec = smallpool.tile([32, 512], F32, tag="rec", bufs=2)
                nc.vector.reciprocal_approx_fast(out=rec[0:1, :], in_=den[:])
                rbc = smallpool.tile([P, 512], F32, tag="rbc", bufs=3)
                base = DH * hh
                nc.vector.stream_shuffle(rbc[base:base + 32, :], rec[:],
                                         [0] * 32)
                nc.vector.stream_shuffle(rbc[base + 32:base + 64, :], rec[:],
                                         [0] * 32)

                def div():
                    nc.gpsimd.tensor_mul(at[rows, sl], at[rows, sl],
                                         rbc[rows, :])
                pending_evac.append(div)
            return emit

        # ---------- build the program ----------
        xn0, xn1 = [], []
        qk0, qk1 = [], []
        vt0, vt1 = [], []
        at_all = {0: [], 1: []}
        xr_all = {0: [], 1: []}

        emit_gn(x0, xn0)

        # head: qkv(0) woven with pair-0 logits
        head = []
        for j in (0, 1, 2, 3):
            for th in range(TH):
                head.append(qk_group(xn0, qk0, j, th, act_evac=(j < 2)))
        wts_cur = {0: [], 1: []}
        lg_p0 = [lg_unit(qk0, 0, hh, st, wts_cur)
                 for hh in range(2) for st in range(ST)]
        mix = [qk_group(xn0, qk0, j, th) for j in (4, 5, 6, 7)
               for th in range(TH)]
        mix += [vt_group(xn0, vt0, st) for st in range(ST)]
        for i, u in enumerate(lg_p0):
            head.append(u)
            head.append(mix[i])
        for u in head:
            u()

        # GN(1) as staged filler units
        gn1_vals = []
        gn1_amu = []

        def gn1_stats_unit(j0, j1):
            def emit():
                gn_stats(x1[j0], gn1_vals)
                gn_stats(x1[j1], gn1_vals)
            return emit

        def gn1_coeffs_unit():
            def emit():
                gn1_amu.append(gn_coeffs(gn1_vals))
            return emit

        def gn1_apply_unit(j):
            def emit():
                xn = xnpool.tile([P, T], BF16, tag="xn")
                xn1.append(xn)
                gn_apply(x1[j], j, gn1_amu[0], xn, eng=nc.gpsimd)
            return emit

        # filler queue: rest of qk(0), GN(1), then qkv(1), then proj work
        fillers = []
        fillers += [gn1_stats_unit(0, 1), gn1_stats_unit(2, 3),
                    gn1_coeffs_unit()]
        fillers += [gn1_apply_unit(j) for j in range(CT)]
        fillers += [qk_group(xn1, qk1, j, th) for j in (0, 1) for th in range(TH)]
        fillers += [vt_group(xn1, vt1, st) for st in range(ST)]
        fillers += [qk_group(xn1, qk1, j, th)
                    for j in (2, 3, 4, 5, 6, 7) for th in range(TH)]

        pair_ids = [(b, p) for b in range(BPC) for p in range(NPAIR)]
        qks = {0: qk0, 1: qk1}
        vts = {0: vt0, 1: vt1}
        proj1_held = []

        def n_fill(slot, group):
            # slots 0-2: 9 fillers (35 qkv/gn units total); slot 3: 8;
            # slots 4+: 1/group (proj(0), then slot-7 proj(1) pre-groups)
            if slot < 3:
                return 3 if group == 0 else 2
            if slot == 3:
                return 2
            return 1

        for idx, (b, p_i) in enumerate(pair_ids):
            vt_sb = vts[b]
            at = attnpool.tile([P, T], BF16, tag="attn", name="at_t")
            at_all[b].append(at)

            wts_next = {0: [], 1: []}
            if idx + 1 < len(pair_ids):
                nb, np_i = pair_ids[idx + 1]
                lg_next = [lg_unit(qks[nb], np_i, hh, st, wts_next)
                           for hh in range(2) for st in range(ST)]
            else:
                lg_next = []

            g_i = 0
            for hh in range(2):
                for th in range(TH):
                    attn_group(vt_sb, wts_cur, at, p_i, hh, th,
                               last=(idx == len(pair_ids) - 1))()
                    # 4 logits units of next pair spaced around a filler
                    mine = lg_next[4 * g_i:4 * g_i + 4]
                    for u in mine[:2]:
                        u()
                    if fillers:
                        fillers.pop(0)()
                    for u in mine[2:]:
                        u()
                    for _ in range(n_fill(idx, g_i) - 1):
                        if fillers:
                            fillers.pop(0)()
                    # deferred divide: one group later (shuffle chain short)
                    if len(pending_evac) > 1:
                        pending_evac.pop(0)()
                    g_i += 1
            wts_cur = wts_next

            # schedule residual reloads + proj fillers
            if b == 0 and p_i == 2:
                for j in range(CT):
                    xr = xrpool.tile([P, T], F32, tag="xr")
                    dma.dma_start(xr[:], xs_d[0, P * j:P * (j + 1), :])
                    xr_all[0].append(xr)
            if b == 0 and p_i == NPAIR - 1:
                # all batch-0 divides must precede any proj(0) reader
                for e in pending_evac:
                    e()
                pending_evac.clear()
                fillers += [proj_group(0, at_all[0], xr_all[0], j, th)
                            for j in range(CT) for th in range(TH)]
            if b == 1 and p_i == 1:
                for j in range(CT):
                    xr = xrpool.tile([P, T], F32, tag="xr")
                    dma.dma_start(xr[:], xs_d[1, P * j:P * (j + 1), :])
                    xr_all[1].append(xr)

            # entering slot 7: pre-open proj(1) groups for j0/j1 with the
            # k=0..1 contraction steps (pairs (1,0)/(1,1): long divided)
            if b == 1 and p_i == 2:
                def pre_unit(j):
                    def emit():
                        ps2 = ps_lg.tile([P, T], F32, tag="pslg")
                        for th in range(TH):
                            ps = ps2[:, 512 * th:512 * (th + 1)]
                            proj_steps(ps, at_all[1], j, th, range(CT - 2))
                            proj1_held.append((ps, j, th))
                    return emit
                fillers += [pre_unit(0), pre_unit(1)]

        # tail: flush divides; k2 steps for held groups (reads pair (1,2));
        # open j2/j3 groups; then all k3 finishers
        for e in pending_evac:
            e()
        pending_evac.clear()
        for g in fillers:
            g()
        for ps, j, th in proj1_held:
            proj_steps(ps, at_all[1], j, th, [CT - 2])
        late = []
        for th in range(TH):
            ps = ps_mm.tile([P, 512], F32, tag="psmm")
            proj_steps(ps, at_all[1], 2, th, range(CT - 1))
            late.append((ps, 2, th))
        for th in range(TH):
            ps = ps_at.tile([P, 512], F32, tag="psat")
            proj_steps(ps, at_all[1], 3, th, range(CT - 1))
            late.append((ps, 3, th))
        for ps, j, th in proj1_held + late:
            proj_fin(1, ps, at_all[1], xr_all[1], j, th)

    nc.compile()
    return nc


def prep_inputs(x, gn_scale, gn_bias, w_qkv, b_qkv, w_proj, b_proj):
    """Host-side: reorder/scale weights, build per-core input maps."""
    x2 = np.ascontiguousarray(
        np.asarray(x, dtype=np.float32).reshape(B, C, T))
    w_qkv = np.asarray(w_qkv, dtype=np.float32)
    b_qkv = np.asarray(b_qkv, dtype=np.float32)
    scale = float(DH) ** -0.25

    qk_rows = []
    for p_i in range(NPAIR):
        for hh in range(2):           # q rows of the pair
            h = 2 * p_i + hh
            qk_rows.extend(range(192 * h, 192 * h + DH))
        for hh in range(2):           # k rows of the pair
            h = 2 * p_i + hh
            qk_rows.extend(range(192 * h + DH, 192 * h + 2 * DH))
    qk_rows = np.array(qk_rows)
    bf16 = mybir.dt.np(BF16)
    wqkT = np.ascontiguousarray((w_qkv[qk_rows] * scale).T).astype(bf16)
    bqkT = np.ascontiguousarray(
        (b_qkv[qk_rows] * scale).reshape(OT, P).T)

    v_rows = np.array([192 * h + 2 * DH + j for h in range(NHEADS)
                       for j in range(DH)])
    wvT = np.ascontiguousarray(w_qkv[v_rows].T).astype(bf16)
    bv = b_qkv[v_rows]
    bvA = np.zeros(NHEADS * 65, np.float32)
    for h in range(NHEADS):
        bvA[65 * h:65 * h + DH] = bv[DH * h:DH * (h + 1)]
        bvA[65 * h + DH] = 1.0

    wpT = np.ascontiguousarray(np.asarray(w_proj, np.float32).T).astype(bf16)
    bpT = np.ascontiguousarray(
        np.asarray(b_proj, np.float32).reshape(CT, P).T)
    gnsb = np.zeros((P, 2 * CT), np.float32)
    gs = np.asarray(gn_scale, np.float32)
    gb = np.asarray(gn_bias, np.float32)
    for j in range(CT):
        gnsb[:, 2 * j] = gs[P * j:P * (j + 1)]
        gnsb[:, 2 * j + 1] = gb[P * j:P * (j + 1)]
    mred = np.zeros((CT, P, NGROUPS), np.float32)
    for j in range(CT):
        for pp in range(P):
            mred[j, pp, (P // GSIZE) * j + pp // GSIZE] = 1.0
    mbc = np.ascontiguousarray(mred.transpose(0, 2, 1))

    common = dict(wqkT=wqkT, bqkT=bqkT, wvT=wvT, bvA=bvA, wpT=wpT,
                  bpT=bpT, gnsb=gnsb, mred=mred, mbc=mbc)
    in_maps = [dict(common, xs=np.ascontiguousarray(x2[BPC * i:BPC * (i + 1)]))
               for i in range(N_CORES)]
    return in_maps


_NC = None


def _ensure_ntff_hook():
    """The agent image's antenv lacks axon_hooks; shim it and register the
    ctypes NTFF hook from the boot script so trace=True can measure HW time."""
    try:
        from antenv import axon_hooks  # noqa: F401
        return
    except ImportError:
        pass
    import types
    import antenv
    mod = types.ModuleType("antenv.axon_hooks")
    _state = {"fn": None}
    mod.set_axon_ntff_profile_hook = lambda fn: _state.__setitem__("fn", fn)
    mod.get_axon_ntff_profile_hook = lambda: _state["fn"]
    sys.modules["antenv.axon_hooks"] = mod
    antenv.axon_hooks = mod
    try:
        from trn_agent_boot.trn_boot import _ntff_profile_via_ctypes
        hook = _ntff_profile_via_ctypes("/opt/axon/libaxon_pjrt.so")
        mod.set_axon_ntff_profile_hook(hook)
    except Exception as e:  # degrade: run proceeds untraced
        print("ntff hook setup failed:", e)


def kernel(x, gn_scale, gn_bias, w_qkv, b_qkv, w_proj, b_proj):
    global _NC, LAST_RESULTS
    if _NC is None:
        _NC = build_nc()
    in_maps = prep_inputs(x, gn_scale, gn_bias, w_qkv, b_qkv, w_proj, b_proj)
    trace = bool(os.environ.get("KERNEL_TRACE"))
    if trace:
        _ensure_ntff_hook()
    res = run_bass_kernel_spmd(_NC, in_maps, list(range(N_CORES)), trace=trace)
    LAST_RESULTS = res
    out = np.concatenate([res.results[i]["out"] for i in range(N_CORES)],
                         axis=0)
    return out.reshape(B, C, HH, WW).astype(np.float32)


# revision 36
# speedup vs baseline: 1.0073x; 1.0073x over previous
"""Trainium2 Bass kernel for nn_AttentionBlock (GroupNorm + MHA + proj + residual).

Sharding: data-parallel over batch (16 batches -> 2 per core x 8 cores).
Weights replicated. Each core computes its 2 batches fully; host gathers.

Per-batch dataflow on a core (c=512, t=1024, H=8 heads, dh=64, 32 groups):
  x [512,1024] -> GroupNorm: bn_stats per tile, tiny mask-matmuls reduce /
      broadcast across partitions (no DRAM bounce), rsqrt via the 0x5f3759df
      bit-trick + 2 Newton steps on DVE (no ACT table switches, ACT keeps
      only Exp) -> xn (bf16), normalizes spread over DVE/Pool/ACT
  qk = Wqk_reordered @ xn   (8 o-tiles; pair-ordered so head-pairs share tiles)
  vT = xn^T @ Wv^T          (v produced transposed: [s, c_v], ones col per head)
  per head-pair: logitsT[s,t] = k^T q (64-row stationary per head)
                 wT = exp(logitsT)  (ScalarE, PSUM->SBUF)
                 attnRaw[65,t] = vAugT^T @ wT  (row 64 = softmax denominator)
                 raw rows copied out at once (frees the PSUM bank);
                 1/denom: copy+reciprocal_approx_fast, partition-broadcast
                 via 2 stream_shuffles, divide applied in-place one group
                 later (on Pool)
  out = w_proj @ attn + b_proj + x  (residual x re-DMA'd from DRAM; proj(1)
        j0/j1 pre-opened with k=0..1 during the last slot, j3 finishes on
        the freed attention PSUM banks; out stores split per t-half)

Schedule: software-pipelined so the PE never idles: per attention group the
next pair's logits (4 units) + filler groups (batch-1 GN/qkv during batch-0
attention, proj afterwards) are woven in. DMA issue order prioritizes
x(batch 0) and Wqk; x1/Wv/Wp follow in first-use order.
"""

import os
import sys

os.environ.setdefault("MYCRO_LOCAL_CACHE", "1")
for _p in ("/root/.axon_site", "/root/.axon_site/_ro/trn_rl_repo",
           "/root/.axon_site/_ro/pypackages", "/opt/trn_rl_repo"):
    if os.path.isdir(_p) and _p not in sys.path:
        sys.path.append(_p)

import numpy as np

from concourse import bass, bacc, tile, mybir
from concourse._compat import get_trn_type
from concourse.bass_utils import run_bass_kernel_spmd

F32 = mybir.dt.float32
BF16 = mybir.dt.bfloat16
MUL = mybir.AluOpType.mult
ADD = mybir.AluOpType.add

N_CORES = 8
B, C, HH, WW = 16, 512, 32, 32
T = HH * WW            # 1024
NHEADS = 8
DH = C // NHEADS       # 64
NGROUPS = 32
GSIZE = C // NGROUPS   # 16 channels per group
EPS = 1e-5
BPC = B // N_CORES     # batches per core = 2
P = 128
NPAIR = NHEADS // 2    # 4 head pairs
CT = C // P            # 4 channel tiles
OT = (2 * C) // P      # 8 qk output tiles
ST = T // P            # 8 s-tiles
TH = T // 512          # 2 t-halves

LAST_RESULTS = None


def _bc_ap(ap, nparts):
    """Broadcast an AP along a new leading partition dim of size nparts."""
    return bass.AP(tensor=ap.tensor, offset=ap.offset,
                   ap=[[0, nparts]] + [list(d) for d in ap.ap])


def build_nc():
    nc = bacc.Bacc(get_trn_type() or "TRN2", target_bir_lowering=False,
                   debug=False)

    xs_d = nc.dram_tensor("xs", [BPC, C, T], F32, kind="ExternalInput")
    wqkT_d = nc.dram_tensor("wqkT", [C, 2 * C], BF16, kind="ExternalInput")
    bqkT_d = nc.dram_tensor("bqkT", [P, OT], F32, kind="ExternalInput")
    wvT_d = nc.dram_tensor("wvT", [C, C], BF16, kind="ExternalInput")
    bvA_d = nc.dram_tensor("bvA", [NHEADS * 65], F32, kind="ExternalInput")
    wpT_d = nc.dram_tensor("wpT", [C, C], BF16, kind="ExternalInput")
    bpT_d = nc.dram_tensor("bpT", [P, CT], F32, kind="ExternalInput")
    gnsb_d = nc.dram_tensor("gnsb", [P, 2 * CT], F32, kind="ExternalInput")
    mred_d = nc.dram_tensor("mred", [CT, P, NGROUPS], F32, kind="ExternalInput")
    mbc_d = nc.dram_tensor("mbc", [CT, NGROUPS, P], F32, kind="ExternalInput")
    out_d = nc.dram_tensor("out", [BPC, C, T], F32, kind="ExternalOutput")
    NGT = P // GSIZE       # groups per 128-channel tile = 8

    from contextlib import ExitStack
    with ExitStack() as ctx:
        tc = ctx.enter_context(tile.TileContext(nc))
        cpool = ctx.enter_context(tc.tile_pool(name="const", bufs=1))
        xpool = ctx.enter_context(tc.tile_pool(name="xp", bufs=5))
        xnpool = ctx.enter_context(tc.tile_pool(name="xnp", bufs=8))
        qkpool = ctx.enter_context(tc.tile_pool(name="qkp", bufs=12))
        vtpool = ctx.enter_context(tc.tile_pool(name="vtp", bufs=16))
        wtpool = ctx.enter_context(tc.tile_pool(name="wtp", bufs=24))
        attnpool = ctx.enter_context(tc.tile_pool(name="attnp", bufs=7))
        outpool = ctx.enter_context(tc.tile_pool(name="outp", bufs=4))
        xrpool = ctx.enter_context(tc.tile_pool(name="xrp", bufs=5))
        smallpool = ctx.enter_context(tc.tile_pool(name="smallp", bufs=3))
        dramp = ctx.enter_context(tc.tile_pool(name="dramp", bufs=2, space="DRAM"))
        ps_mm = ctx.enter_context(tc.tile_pool(name="ps_mm", bufs=2, space="PSUM"))
        ps_lg = ctx.enter_context(tc.tile_pool(name="ps_lg", bufs=2, space="PSUM"))
        ps_at = ctx.enter_context(tc.tile_pool(name="ps_at", bufs=2, space="PSUM"))

        dma = nc.default_dma_engine

        def load_x(b, split=1):
            xs = []
            for j in range(CT):
                xt = xpool.tile([P, T], F32, tag="x")
                w = T // split
                for c in range(split):
                    dma.dma_start(xt[:, w * c:w * (c + 1)],
                                  xs_d[b, P * j:P * (j + 1),
                                       w * c:w * (c + 1)])
                xs.append(xt)
            return xs

        # DMA issue order = SP queue order: x0 first, GN consts, qk weights,
        # then x1 and the rest (wpT last: needed only at proj time)
        x0 = load_x(0, split=2)
        gnsb_sb = cpool.tile([P, 2 * CT], F32, tag="gnsb")
        dma.dma_start(gnsb_sb[:], gnsb_d[:])
        mred4_sb, mbc4_sb = [], []
        for j in range(CT):
            m = cpool.tile([P, NGROUPS], F32, tag=f"mred{j}")
            dma.dma_start(m[:], mred_d[j])
            mred4_sb.append(m)
            m2 = cpool.tile([NGROUPS, P], F32, tag=f"mbc{j}")
            dma.dma_start(m2[:], mbc_d[j])
            mbc4_sb.append(m2)
        wqkT_sb = []
        for k in range(CT):
            w = cpool.tile([P, 2 * C], BF16, tag=f"wqkT{k}")
            dma.dma_start(w[:], wqkT_d[P * k:P * (k + 1), :])
            wqkT_sb.append(w)
        bqk_sb = cpool.tile([P, OT], F32, tag="bqk")
        dma.dma_start(bqk_sb[:], bqkT_d[:])
        wvT_sb = []
        for k in range(CT):
            w = cpool.tile([P, C], BF16, tag=f"wvT{k}")
            dma.dma_start(w[:], wvT_d[P * k:P * (k + 1), :])
            wvT_sb.append(w)
        bv_bc = cpool.tile([P, NHEADS * 65], F32, tag="bv")
        dma.dma_start(bv_bc[:], _bc_ap(bvA_d.ap(), P))
        x1 = load_x(1, split=2)
        bp_sb = cpool.tile([P, CT], F32, tag="bp")
        dma.dma_start(bp_sb[:], bpT_d[:])
        wpT_sb = []
        for k in range(CT):
            w = cpool.tile([P, C], BF16, tag=f"wpT{k}")
            dma.dma_start(w[:], wpT_d[P * k:P * (k + 1), :])
            wpT_sb.append(w)
        zero_b = cpool.tile([P, 1], F32, tag="zerob")
        nc.vector.memset(zero_b[:], 0.0)
        magic_sb = cpool.tile([NGROUPS, 1], mybir.dt.int32, tag="magic")
        nc.vector.memset(magic_sb[:], 0x5F3759DF)
        c15_sb = cpool.tile([NGROUPS, 1], F32, tag="c15")
        nc.vector.memset(c15_sb[:], 1.5)

        # ---------- GroupNorm: combined 32-group stage, all-DVE rsqrt ------
        I32 = mybir.dt.int32

        def gn_stats(xt, vals_list):
            bst = smallpool.tile([P, 2, 6], F32, tag="bst")
            nc.vector.bn_stats(out=bst[:, 0, :], in_=xt[:, 0:512])
            nc.vector.bn_stats(out=bst[:, 1, :], in_=xt[:, 512:1024])
            mv = smallpool.tile([P, 3], F32, tag="mv", bufs=5)
            nc.vector.bn_aggr(out=mv[:, 0:2], in_=bst[:])
            nc.vector.tensor_mul(mv[:, 2:3], mv[:, 0:1], mv[:, 0:1])
            vals_list.append(mv)

        def gn_coeffs(vals_list):
            """Reduce all 32 groups, rsqrt via bit-trick + 2 Newton (DVE only).
            Returns amu [32, 2] = (a_g, mu_g)."""
            gp = ps_mm.tile([P, 512], F32, tag="psmm")
            for j in range(CT):
                nc.tensor.matmul(gp[0:NGROUPS, 0:3], mred4_sb[j][:],
                                 vals_list[j][:],
                                 start=(j == 0), stop=(j == CT - 1))
            gs = smallpool.tile([NGROUPS, 3], F32, tag="gs", bufs=3)
            nc.vector.tensor_copy(gs[:], gp[0:NGROUPS, 0:3])
            gg = smallpool.tile([NGROUPS, 7], F32, tag="gg", bufs=3)
            # cols: 0=mu 1=sum(var)+sum(mu^2) 2=v(+eps) 3=h 5=u 6=e
            nc.vector.tensor_scalar_mul(gg[:, 0:1], gs[:, 0:1], 1.0 / GSIZE)
            nc.vector.tensor_add(gg[:, 1:2], gs[:, 1:2], gs[:, 2:3])
            nc.vector.tensor_mul(gg[:, 2:3], gg[:, 0:1], gg[:, 0:1])
            nc.vector.scalar_tensor_tensor(
                out=gg[:, 2:3], in0=gg[:, 1:2], scalar=1.0 / GSIZE,
                in1=gg[:, 2:3], op0=MUL, op1=mybir.AluOpType.subtract)
            nc.vector.tensor_scalar_add(gg[:, 2:3], gg[:, 2:3], EPS)
            nc.vector.tensor_scalar_mul(gg[:, 3:4], gg[:, 2:3], 0.5)  # h=v/2
            gi = smallpool.tile([NGROUPS, 1], I32, tag="gi")
            nc.vector.tensor_scalar(
                out=gi[:], in0=gg[:, 2:3].bitcast(I32), scalar1=1,
                scalar2=None, op0=mybir.AluOpType.logical_shift_right)
            amu = smallpool.tile([NGROUPS, 2], F32, tag="amu")
            y = amu[:, 0:1]
            nc.vector.tensor_sub(y.bitcast(I32), magic_sb[:], gi[:])
            for _ in range(2):  # y *= 1.5 - h*y^2
                nc.vector.tensor_mul(gg[:, 5:6], y, y)
                nc.vector.tensor_mul(gg[:, 5:6], gg[:, 5:6], gg[:, 3:4])
                nc.vector.tensor_sub(gg[:, 6:7], c15_sb[:], gg[:, 5:6])
                nc.vector.tensor_mul(y, y, gg[:, 6:7])
            nc.vector.tensor_copy(amu[:, 1:2], gg[:, 0:1])
            return amu

        def gn_abj(j, amu):
            bp2 = ps_mm.tile([P, 512], F32, tag="psmm")
            nc.tensor.matmul(bp2[:, 0:2], mbc4_sb[j][:], amu[:],
                             start=True, stop=True)
            abj = smallpool.tile([P, 2], F32, tag="abj", bufs=6)
            nc.vector.tensor_mul(abj[:, 0:1], bp2[:, 0:1],
                                 gnsb_sb[:, 2 * j:2 * j + 1])
            tmpm = smallpool.tile([P, 1], F32, tag="tmpm")
            nc.vector.tensor_mul(tmpm[:], bp2[:, 1:2], abj[:, 0:1])
            nc.vector.tensor_sub(abj[:, 1:2],
                                 gnsb_sb[:, 2 * j + 1:2 * j + 2], tmpm[:])
            return abj

        def gn_norm(xt, abj, xn_out, eng):
            if eng is nc.scalar:
                nc.scalar.activation(
                    xn_out[:], xt[:], mybir.ActivationFunctionType.Identity,
                    bias=abj[:, 1:2], scale=abj[:, 0:1])
            else:
                eng.tensor_scalar(
                    out=xn_out[:], in0=xt[:], scalar1=abj[:, 0:1],
                    scalar2=abj[:, 1:2], op0=MUL, op1=ADD)

        def gn_apply(xt, j, amu, xn_out, eng=None):
            gn_norm(xt, gn_abj(j, amu), xn_out, eng or nc.vector)

        def emit_gn(x_sb, xn_sb):
            vals_list = []
            for j in range(CT):
                gn_stats(x_sb[j], vals_list)
            amu = gn_coeffs(vals_list)
            abjs = [gn_abj(j, amu) for j in range(CT)]
            engs = [nc.vector, nc.gpsimd, nc.scalar, nc.gpsimd]
            for j in range(CT):
                xn = xnpool.tile([P, T], BF16, tag="xn")
                xn_sb.append(xn)
                gn_norm(x_sb[j], abjs[j], xn, engs[j])

        # ---------- qkv / proj group emitters (one PSUM group each) ----------
        def qk_group(xn_sb, qk_sb, j, th, act_evac=False):
            def emit():
                if th == 0:
                    qk_sb.append(qkpool.tile([P, T], BF16, tag="qk",
                                             name="qk_t"))
                qk = qk_sb[j]
                ps = ps_mm.tile([P, 512], F32, tag="psmm")
                for k in range(CT):
                    nc.tensor.matmul(
                        ps[:], wqkT_sb[k][:, P * j:P * (j + 1)],
                        xn_sb[k][:, 512 * th:512 * (th + 1)],
                        start=(k == 0), stop=(k == CT - 1))
                if act_evac:
                    nc.scalar.activation(
                        qk[:, 512 * th:512 * (th + 1)], ps[:],
                        mybir.ActivationFunctionType.Identity,
                        bias=bqk_sb[:, j:j + 1])
                else:
                    nc.vector.tensor_scalar_add(
                        qk[:, 512 * th:512 * (th + 1)], ps[:],
                        bqk_sb[:, j:j + 1])
            return emit

        def vt_group(xn_sb, vt_sb, st):
            def emit():
                vt = vtpool.tile([P, NHEADS * 65], BF16, tag="vt")
                vt_sb.append(vt)
                vt3 = vt[:].rearrange("p (h c) -> p h c", h=NHEADS)
                ps = ps_mm.tile([P, 512], F32, tag="psmm")
                for k in range(CT):
                    nc.tensor.matmul(
                        ps[:], xn_sb[k][:, P * st:P * (st + 1)], wvT_sb[k][:],
                        start=(k == 0), stop=(k == CT - 1))
                bv3 = bv_bc[:].rearrange("p (h c) -> p h c", h=NHEADS)
                nc.vector.tensor_add(
                    vt3[:, :, 0:DH],
                    ps[:].rearrange("p (h c) -> p h c", h=NHEADS),
                    bv3[:, :, 0:DH])
                nc.gpsimd.tensor_copy(vt3[:, :, DH:DH + 1],
                                      bv3[:, :, DH:DH + 1])
            return emit

        def proj_steps(ps, at_sb, j, th, ks, stop=False):
            for k in ks:
                nc.tensor.matmul(
                    ps[:], wpT_sb[k][:, P * j:P * (j + 1)],
                    at_sb[k][:, 512 * th:512 * (th + 1)],
                    start=(k == 0), stop=(stop and k == CT - 1))

        def proj_fin(b, ps, at_sb, xr_sb, j, th):
            proj_steps(ps, at_sb, j, th, [CT - 1], stop=True)
            ot = outpool.tile([P, 512], F32, tag="out", name="out_t")
            nc.vector.scalar_tensor_tensor(
                out=ot[:], in0=ps[:], scalar=bp_sb[:, j:j + 1],
                in1=xr_sb[j][:, 512 * th:512 * (th + 1)],
                op0=ADD, op1=ADD)
            dma.dma_start(
                out_d[b, P * j:P * (j + 1), 512 * th:512 * (th + 1)], ot[:])

        def proj_group(b, at_sb, xr_sb, j, th):
            def emit():
                ps = ps_mm.tile([P, 512], F32, tag="psmm")
                proj_steps(ps, at_sb, j, th, range(CT - 1))
                proj_fin(b, ps, at_sb, xr_sb, j, th)
            return emit

        # ---------- attention emitters ----------
        def lg_unit(qk_sb, p_i, hh, st, wts):
            """Logits (2 MMs) + exp for one (head, s-tile). Appends wt tile."""
            def emit():
                qt = qk_sb[2 * p_i]
                kt = qk_sb[2 * p_i + 1]
                lo = DH * hh
                lg = ps_lg.tile([P, T], F32, tag="pslg")
                for th in range(TH):
                    nc.tensor.matmul(
                        lg[:, 512 * th:512 * (th + 1)],
                        kt[lo:lo + DH, P * st:P * (st + 1)],
                        qt[lo:lo + DH, 512 * th:512 * (th + 1)],
                        start=True, stop=True)
                wt = wtpool.tile([P, T], BF16, tag="wt")
                nc.scalar.activation(wt[:], lg[:],
                                     mybir.ActivationFunctionType.Exp,
                                     bias=zero_b[:])
                wts[hh].append(wt)
            return emit

        pending_evac = []

        def attn_group(vt_sb, wts, at, p_i, hh, th, last=False):
            """attn-MM group; raw rows copied out at once (frees the PSUM
            bank), 1/denom broadcast via gpsimd DRAM bounce, divide applied
            in-place later on Pool (pending_evac). For the final pair the
            raw/den copies run on the tail-idle ACT engine instead of DVE."""
            def emit():
                h_abs = 2 * p_i + hh
                pa = ps_at.tile([65, 512], F32, tag="psat")
                for st in range(ST):
                    nc.tensor.matmul(
                        pa[:], vt_sb[st][:, 65 * h_abs:65 * (h_abs + 1)],
                        wts[hh][st][:, 512 * th:512 * (th + 1)],
                        start=(st == 0), stop=(st == ST - 1))
                sl = slice(512 * th, 512 * (th + 1))
                rows = slice(DH * hh, DH * hh + DH)
                den = smallpool.tile([1, 512], F32, tag="den", bufs=2)
                if last:
                    nc.scalar.copy(den[:], pa[DH:DH + 1, :])
                    nc.scalar.copy(at[rows, sl], pa[0:DH, :])
                else:
                    nc.vector.tensor_copy(den[:], pa[DH:DH + 1, :])
                    nc.vector.tensor_copy(at[rows, sl], pa[0:DH, :])
                rec = smallpool.tile([32, 512], F32, tag="rec", bufs=2)
                nc.vector.reciprocal_approx_fast(out=rec[0:1, :], in_=den[:])
                rbc = smallpool.tile([P, 512], F32, tag="rbc", bufs=3)
                base = DH * hh
                nc.vector.stream_shuffle(rbc[base:base + 32, :], rec[:],
                                         [0] * 32)
                nc.vector.stream_shuffle(rbc[base + 32:base + 64, :], rec[:],
                                         [0] * 32)

                def div():
                    nc.gpsimd.tensor_mul(at[rows, sl], at[rows, sl],
                                         rbc[rows, :])
                pending_evac.append(div)
            return emit

        # ---------- build the program ----------
        xn0, xn1 = [], []
        qk0, qk1 = [], []
        vt0, vt1 = [], []
        at_all = {0: [], 1: []}
        xr_all = {0: [], 1: []}

        emit_gn(x0, xn0)

        # head: qkv(0) woven with pair-0 logits
        head = []
        for j in (0, 1, 2, 3):
            for th in range(TH):
                head.append(qk_group(xn0, qk0, j, th, act_evac=(j < 2)))
        wts_cur = {0: [], 1: []}
        lg_p0 = [lg_unit(qk0, 0, hh, st, wts_cur)
                 for hh in range(2) for st in range(ST)]
        mix = [qk_group(xn0, qk0, j, th) for j in (4, 5, 6, 7)
               for th in range(TH)]
        mix += [vt_group(xn0, vt0, st) for st in range(ST)]
        for i, u in enumerate(lg_p0):
            head.append(u)
            head.append(mix[i])
        for u in head:
            u()

        # GN(1) as staged filler units
        gn1_vals = []
        gn1_amu = []

        def gn1_stats_unit(j0, j1):
            def emit():
                gn_stats(x1[j0], gn1_vals)
                gn_stats(x1[j1], gn1_vals)
            return emit

        def gn1_coeffs_unit():
            def emit():
                gn1_amu.append(gn_coeffs(gn1_vals))
            return emit

        def gn1_apply_unit(j):
            def emit():
                xn = xnpool.tile([P, T], BF16, tag="xn")
                xn1.append(xn)
                gn_apply(x1[j], j, gn1_amu[0], xn, eng=nc.gpsimd)
            return emit

        # filler queue: rest of qk(0), GN(1), then qkv(1), then proj work
        fillers = []
        fillers += [gn1_stats_unit(0, 1), gn1_stats_unit(2, 3),
                    gn1_coeffs_unit()]
        fillers += [gn1_apply_unit(j) for j in range(CT)]
        fillers += [qk_group(xn1, qk1, j, th) for j in (0, 1) for th in range(TH)]
        fillers += [vt_group(xn1, vt1, st) for st in range(ST)]
        fillers += [qk_group(xn1, qk1, j, th)
                    for j in (2, 3, 4, 5, 6, 7) for th in range(TH)]

        pair_ids = [(b, p) for b in range(BPC) for p in range(NPAIR)]
        qks = {0: qk0, 1: qk1}
        vts = {0: vt0, 1: vt1}
        proj1_held = []

        def n_fill(slot, group):
            # slots 0-2: 9 fillers (35 qkv/gn units total); slot 3: 8;
            # slots 4+: 1/group (proj(0), then slot-7 proj(1) pre-groups)
            if slot < 3:
                return 3 if group == 0 else 2
            if slot == 3:
                return 2
            return 1

        for idx, (b, p_i) in enumerate(pair_ids):
            vt_sb = vts[b]
            at = attnpool.tile([P, T], BF16, tag="attn", name="at_t")
            at_all[b].append(at)

            wts_next = {0: [], 1: []}
            if idx + 1 < len(pair_ids):
                nb, np_i = pair_ids[idx + 1]
                lg_next = [lg_unit(qks[nb], np_i, hh, st, wts_next)
                           for hh in range(2) for st in range(ST)]
            else:
                lg_next = []

            g_i = 0
            for hh in range(2):
                for th in range(TH):
                    attn_group(vt_sb, wts_cur, at, p_i, hh, th,
                               last=(idx == len(pair_ids) - 1))()
                    # 4 logits units of next pair spaced around a filler
                    mine = lg_next[4 * g_i:4 * g_i + 4]
                    for u in mine[:2]:
                        u()
                    if fillers:
                        fillers.pop(0)()
                    for u in mine[2:]:
                        u()
                    for _ in range(n_fill(idx, g_i) - 1):
                        if fillers:
                            fillers.pop(0)()
                    # deferred divide: one group later (shuffle chain short)
                    if len(pending_evac) > 1:
                        pending_evac.pop(0)()
                    g_i += 1
            wts_cur = wts_next

            # schedule residual reloads + proj fillers
            if b == 0 and p_i == 2:
                for j in range(CT):
                    xr = xrpool.tile([P, T], F32, tag="xr")
                    dma.dma_start(xr[:], xs_d[0, P * j:P * (j + 1), :])
                    xr_all[0].append(xr)
            if b == 0 and p_i == NPAIR - 1:
                # all batch-0 divides must precede any proj(0) reader
                for e in pending_evac:
                    e()
                pending_evac.clear()
                fillers += [proj_group(0, at_all[0], xr_all[0], j, th)
                            for j in range(CT) for th in range(TH)]
            if b == 1 and p_i == 1:
                for j in range(CT):
                    xr = xrpool.tile([P, T], F32, tag="xr")
                    dma.dma_start(xr[:], xs_d[1, P * j:P * (j + 1), :])
                    xr_all[1].append(xr)

            # entering slot 7: pre-open proj(1) groups for j0/j1 with the
            # k=0..1 contraction steps (pairs (1,0)/(1,1): long divided)
            if b == 1 and p_i == 2:
                def pre_unit(j):
                    def emit():
                        ps2 = ps_lg.tile([P, T], F32, tag="pslg")
                        for th in range(TH):
                            ps = ps2[:, 512 * th:512 * (th + 1)]
                            proj_steps(ps, at_all[1], j, th, range(CT - 2))
                            proj1_held.append((ps, j, th))
                    return emit
                fillers += [pre_unit(0), pre_unit(1)]

        # tail: flush divides; k2 steps for held groups (reads pair (1,2));
        # open j2/j3 groups; then all k3 finishers
        for e in pending_evac:
            e()
        pending_evac.clear()
        for g in fillers:
            g()
        for ps, j, th in proj1_held:
            proj_steps(ps, at_all[1], j, th, [CT - 2])
        late = []
        for th in range(TH):
            ps = ps_mm.tile([P, 512], F32, tag="psmm")
            proj_steps(ps, at_all[1], 2, th, range(CT - 1))
            late.append((ps, 2, th))
        for th in range(TH):
            ps = ps_at.tile([P, 512], F32, tag="psat")
            proj_steps(ps, at_all[1], 3, th, range(CT - 1))
            late.append((ps, 3, th))
        for ps, j, th in proj1_held + late:
            proj_fin(1, ps, at_all[1], xr_all[1], j, th)

    nc.compile()
    return nc


def prep_inputs(x, gn_scale, gn_bias, w_qkv, b_qkv, w_proj, b_proj):
    """Host-side: reorder/scale weights, build per-core input maps."""
    x2 = np.ascontiguousarray(
        np.asarray(x, dtype=np.float32).reshape(B, C, T))
    w_qkv = np.asarray(w_qkv, dtype=np.float32)
    b_qkv = np.asarray(b_qkv, dtype=np.float32)
    scale = float(DH) ** -0.25

    qk_rows = []
    for p_i in range(NPAIR):
        for hh in range(2):           # q rows of the pair
            h = 2 * p_i + hh
            qk_rows.extend(range(192 * h, 192 * h + DH))
        for hh in range(2):           # k rows of the pair
            h = 2 * p_i + hh
            qk_rows.extend(range(192 * h + DH, 192 * h + 2 * DH))
    qk_rows = np.array(qk_rows)
    bf16 = mybir.dt.np(BF16)
    wqkT = np.ascontiguousarray((w_qkv[qk_rows] * scale).T).astype(bf16)
    bqkT = np.ascontiguousarray(
        (b_qkv[qk_rows] * scale).reshape(OT, P).T)

    v_rows = np.array([192 * h + 2 * DH + j for h in range(NHEADS)
                       for j in range(DH)])
    wvT = np.ascontiguousarray(w_qkv[v_rows].T).astype(bf16)
    bv = b_qkv[v_rows]
    bvA = np.zeros(NHEADS * 65, np.float32)
    for h in range(NHEADS):
        bvA[65 * h:65 * h + DH] = bv[DH * h:DH * (h + 1)]
        bvA[65 * h + DH] = 1.0

    wpT = np.ascontiguousarray(np.asarray(w_proj, np.float32).T).astype(bf16)
    bpT = np.ascontiguousarray(
        np.asarray(b_proj, np.float32).reshape(CT, P).T)
    gnsb = np.zeros((P, 2 * CT), np.float32)
    gs = np.asarray(gn_scale, np.float32)
    gb = np.asarray(gn_bias, np.float32)
    for j in range(CT):
        gnsb[:, 2 * j] = gs[P * j:P * (j + 1)]
        gnsb[:, 2 * j + 1] = gb[P * j:P * (j + 1)]
    mred = np.zeros((CT, P, NGROUPS), np.float32)
    for j in range(CT):
        for pp in range(P):
            mred[j, pp, (P // GSIZE) * j + pp // GSIZE] = 1.0
    mbc = np.ascontiguousarray(mred.transpose(0, 2, 1))

    common = dict(wqkT=wqkT, bqkT=bqkT, wvT=wvT, bvA=bvA, wpT=wpT,
                  bpT=bpT, gnsb=gnsb, mred=mred, mbc=mbc)
    in_maps = [dict(common, xs=np.ascontiguousarray(x2[BPC * i:BPC * (i + 1)]))
               for i in range(N_CORES)]
    return in_maps


_NC = None


def _ensure_ntff_hook():
    """The agent image's antenv lacks axon_hooks; shim it and register the
    ctypes NTFF hook from the boot script so trace=True can measure HW time."""
    try:
        from antenv import axon_hooks  # noqa: F401
        return
    except ImportError:
        pass
    import types
    import antenv
    mod = types.ModuleType("antenv.axon_hooks")
    _state = {"fn": None}
    mod.set_axon_ntff_profile_hook = lambda fn: _state.__setitem__("fn", fn)
    mod.get_axon_ntff_profile_hook = lambda: _state["fn"]
    sys.modules["antenv.axon_hooks"] = mod
    antenv.axon_hooks = mod
    try:
        from trn_agent_boot.trn_boot import _ntff_profile_via_ctypes
        hook = _ntff_profile_via_ctypes("/opt/axon/libaxon_pjrt.so")
        mod.set_axon_ntff_profile_hook(hook)
    except Exception as e:  # degrade: run proceeds untraced
        print("ntff hook setup failed:", e)


def kernel(x, gn_scale, gn_bias, w_qkv, b_qkv, w_proj, b_proj):
    global _NC, LAST_RESULTS
    if _NC is None:
        _NC = build_nc()
    in_maps = prep_inputs(x, gn_scale, gn_bias, w_qkv, b_qkv, w_proj, b_proj)
    trace = bool(os.environ.get("KERNEL_TRACE"))
    if trace:
        _ensure_ntff_hook()
    res = run_bass_kernel_spmd(_NC, in_maps, list(range(N_CORES)), trace=trace)
    LAST_RESULTS = res
    out = np.concatenate([res.results[i]["out"] for i in range(N_CORES)],
                         axis=0)
    return out.reshape(B, C, HH, WW).astype(np.float32)
